# revision 15
# baseline (speedup 1.0000x reference)
"""Trainium2 Bass kernel for a discriminative (instance-segmentation) loss.

Math (per batch b, with E=64-dim embeddings, K=32 clusters, N=4096 points):
  centroids C[k] = sum_n masks[n,k]*emb[n] / msum[k]
  L_v = mean_b sum_n relu(||emb_n - C_own(n)|| - 0.5)^2 / N
  L_d = mean_b sum_{k!=j} relu(3 - ||C_k - C_j||)^2 / (K*(K-1))
  L_r = mean_b mean_k ||C_k||
  loss = L_v + L_d + 0.001 * L_r

Sharding: data-parallel over the batch dim (B=8 -> 8 NeuronCores, one batch
each).  Each core computes its per-batch scalar; the host averages the 8
scalars.

Per-core layout: n = 32*p + c  (p = SBUF partition 0..127, c = chunk 0..31),
so each partition's slice of `emb`/`masks` is one contiguous DRAM block
(line-rate DMA descriptors).  Chunks are processed in 8 groups of 4.

Pipeline per core:
  1. PE: Cu^T[e,k]  = sum_c emb_c^T @ masks_c            (32 accum. matmuls)
  2. PE: masksT groups via matmul against [I_128 | 1]    (8 matmuls; the
     extra ones-column yields per-group cluster-count partials for msum)
  3. msum -> 1/msum; normalize Cu -> C; cn2[k] = ||C_k||^2
  4. PE: C_own per group via block-diag trick:
     lhsT = masksT_g [128,128], rhs = blockdiag(C,C,C,C) [128,256]
     -> psum[p, 64j+e] = C_own(n=32p+4g+j, e)
  5. DVE/ACT: diff = C_own - emb; dist2 = sum_e diff^2;
     hv = relu(sqrt(dist2)-0.5); row-accumulate hv^2/N
  6. tiny [32,32] centroid-pairwise hinge + mean-norm tail
  7. one final matmul against ones -> scalar -> DMA out
"""

from contextlib import ExitStack

import numpy as np

import concourse.bass as bass
import concourse.bacc as bacc
import concourse.tile as tile
from concourse import mybir
from concourse import bass_utils

F32 = mybir.dt.float32
AX = mybir.AxisListType
OP = mybir.AluOpType
AF = mybir.ActivationFunctionType

B, N, E, K = 8, 4096, 64, 32
P = 128            # SBUF partitions; n = 32*p + c
CHUNKS = N // P    # 32
GROUPS = 8         # 4 chunks per group
CPG = CHUNKS // GROUPS  # 4
DELTA_V = 0.5
DELTA_D = 1.5
ALPHA, BETA, GAMMA = 1.0, 1.0, 0.001


def _body(nc, tc, ctx, t, stage):
    """Emit the kernel body. `stage` < 99 stops early and DMAs an
    intermediate to the debug output (bisection aid)."""
    consts = ctx.enter_context(tc.tile_pool(name="consts", bufs=1))
    big = ctx.enter_context(tc.tile_pool(name="big", bufs=1))
    work = ctx.enter_context(tc.tile_pool(name="work", bufs=3))
    small = ctx.enter_context(tc.tile_pool(name="small", bufs=1))
    p_cu = ctx.enter_context(tc.tile_pool(name="p_cu", bufs=1, space="PSUM"))
    p_mt = ctx.enter_context(tc.tile_pool(name="p_mt", bufs=2, space="PSUM"))
    p_2 = ctx.enter_context(tc.tile_pool(name="p_2", bufs=3, space="PSUM"))
    p_sm = ctx.enter_context(tc.tile_pool(name="p_sm", bufs=2, space="PSUM"))

    def dbg(ap):
        rows, cols = ap.shape[0], int(np.prod(ap.shape[1:]))
        flat = ap if len(ap.shape) == 2 else ap.rearrange(
            "p ... -> p (...)"
        )
        nc.sync.dma_start(out=t["dbg"][0:rows, 0:cols], in_=flat)

    # ---- constants / memsets (off critical path) ----
    id129 = consts.tile([P, P + 1], F32)
    nc.sync.dma_start(out=id129, in_=t["id129"][:, :])
    stki = consts.tile([P, K], F32)
    nc.sync.dma_start(out=stki, in_=t["stki"][:, :])
    eyec = consts.tile([K, K], F32)
    nc.sync.dma_start(out=eyec, in_=t["eyec"][:, :])
    ones1 = consts.tile([P, 1], F32)
    nc.vector.memset(ones1, 1.0)
    bias_m = consts.tile([K, 1], F32)     # 2*DELTA_D margin bias
    nc.vector.memset(bias_m, 2.0 * DELTA_D)
    bias_v = consts.tile([P, 1], F32)     # -DELTA_V hinge bias
    nc.vector.memset(bias_v, -DELTA_V)
    c4bd = big.tile([P, CPG * E], F32)   # blockdiag(C x4), filled later
    nc.vector.memset(c4bd, 0.0)
    init_acc = small.tile([P, 1], F32)   # hinge-sum init vector
    nc.vector.memset(init_acc, 0.0)

    # ---- input loads, one DMA per group (contiguous 1KiB descriptors) ----
    emb_sb = big.tile([P, CHUNKS * E], F32)       # [p, 64*c + e]
    msk_sb = big.tile([P, CHUNKS, K], F32)        # [p, c, k]
    emb_r = t["emb"][:, :].rearrange("(p c) e -> p (c e)", p=P)
    msk_r = t["msk"][:, :].rearrange("(p c) k -> p c k", p=P)
    for g in range(GROUPS):
        c0, c1 = g * CPG, (g + 1) * CPG
        nc.sync.dma_start(out=msk_sb[:, c0:c1, :], in_=msk_r[:, c0:c1, :])
        nc.sync.dma_start(
            out=emb_sb[:, c0 * E:c1 * E], in_=emb_r[:, c0 * E:c1 * E]
        )
    if stage <= 1:
        return dbg(msk_sb[:, 0:4, :])

    # ---- phase 1: masks transposes (+msum partials) ----
    mskT = big.tile([P, GROUPS, P + 1], F32)  # [32j+k, g, p | msum partial]
    for g in range(GROUPS):
        mview = msk_sb[:, g * CPG:(g + 1) * CPG, :].rearrange("p a b -> p (a b)")
        pt = p_mt.tile([P, P + 1], F32)
        nc.tensor.matmul(pt, lhsT=mview, rhs=id129, start=True, stop=True)
        nc.scalar.copy(out=mskT[:, g, :], in_=pt)
    if stage <= 2:
        return dbg(mskT[:, 0, :])

    # ---- Cu^T accumulation ----
    cu_psum = p_cu.tile([E, K], F32)
    for c in range(CHUNKS):
        nc.tensor.matmul(
            cu_psum,
            lhsT=emb_sb[:, c * E:(c + 1) * E],
            rhs=msk_sb[:, c, :],
            start=(c == 0),
            stop=(c == CHUNKS - 1),
        )
    cuT_sb = small.tile([E, K], F32)
    nc.scalar.copy(out=cuT_sb, in_=cu_psum)
    if stage <= 3:
        return dbg(cuT_sb)

    # ---- msum -> recip ----
    msum_parts = mskT[:, :, P:P + 1].rearrange("p g o -> p (g o)")  # [P, 8]
    msum_big = small.tile([P, 1], F32)
    nc.vector.reduce_sum(out=msum_big, in_=msum_parts, axis=AX.X)
    ms_psum = p_sm.tile([K, 1], F32, tag="sm")
    nc.tensor.matmul(ms_psum, lhsT=stki, rhs=msum_big, start=True, stop=True)
    recip = small.tile([K, 1], F32)
    nc.vector.reciprocal(recip, ms_psum)
    if stage <= 4:
        return dbg(recip)

    # ---- C = Cu^T^T * recip; cn2 ----
    c_psum = p_sm.tile([K, E], F32, tag="sm")
    nc.tensor.matmul(c_psum, lhsT=cuT_sb, rhs=id129[0:E, 0:E],
                     start=True, stop=True)
    if stage == 41:
        tmp41 = small.tile([K, E], F32)
        nc.scalar.copy(tmp41, c_psum)
        return dbg(tmp41)
    c_sb = small.tile([K, E], F32)
    nc.vector.tensor_scalar_mul(c_sb, in0=c_psum, scalar1=recip)
    if stage == 42:
        return dbg(c_sb)
    # NOTE: InstTensorTensorReduce crashes the device on this path --
    # use separate mul + reduce instead.
    scr_ke = small.tile([K, E], F32)
    cn2 = small.tile([K, 1], F32)
    nc.vector.tensor_mul(scr_ke, c_sb, c_sb)
    nc.vector.reduce_sum(out=cn2, in_=scr_ke, axis=AX.X)
    if stage <= 5:
        return dbg(c_sb)

    # ---- block-diag C ----
    for j in range(CPG):
        nc.gpsimd.dma_start(
            out=c4bd[j * K:(j + 1) * K, j * E:(j + 1) * E], in_=c_sb
        )
    if stage <= 6:
        return dbg(c4bd)

    # ---- tiny pairwise-centroid tail (L_d, L_r) ----
    gu_psum = p_sm.tile([K, K], F32, tag="sm")
    nc.tensor.matmul(gu_psum, lhsT=cuT_sb, rhs=cuT_sb, start=True, stop=True)
    recipm2 = small.tile([K, 1], F32)
    nc.scalar.mul(recipm2, recip, -2.0)
    x_sb = small.tile([K, K], F32)
    nc.vector.tensor_scalar_mul(x_sb, in0=gu_psum, scalar1=recipm2)
    xt_psum = p_sm.tile([K, K], F32, tag="sm")
    nc.tensor.matmul(xt_psum, lhsT=x_sb, rhs=id129[0:K, 0:K],
                     start=True, stop=True)
    z_sb = small.tile([K, K], F32)
    nc.vector.tensor_scalar(
        out=z_sb, in0=xt_psum, scalar1=recip, scalar2=cn2,
        op0=OP.mult, op1=OP.add,
    )
    zt_psum = p_sm.tile([K, K], F32, tag="sm")
    nc.tensor.matmul(zt_psum, lhsT=z_sb, rhs=id129[0:K, 0:K],
                     start=True, stop=True)
    d2_sb = small.tile([K, K], F32)
    nc.vector.tensor_scalar(
        out=d2_sb, in0=zt_psum, scalar1=cn2, scalar2=0.0,
        op0=OP.add, op1=OP.max,
    )
    d_sb = small.tile([K, K], F32)
    nc.scalar.sqrt(d_sb, d2_sb)
    h0_sb = small.tile([K, K], F32)
    nc.scalar.activation(h0_sb, d_sb, AF.Relu, bias=bias_m, scale=-1.0)
    h_sb = small.tile([K, K], F32)
    nc.vector.tensor_mul(h_sb, h0_sb, eyec)
    scr_kk = small.tile([K, K], F32)
    ld_raw = small.tile([K, 1], F32)
    nc.vector.tensor_mul(scr_kk, h_sb, h_sb)
    nc.vector.reduce_sum(out=ld_raw, in_=scr_kk, axis=AX.X)
    cr_row = small.tile([K, 1], F32)
    nc.scalar.activation(cr_row, cn2, AF.Sqrt, scale=(GAMMA / K) ** 2)
    nc.vector.tensor_scalar(
        out=init_acc[0:K, :], in0=ld_raw, scalar1=BETA / float(K * (K - 1)),
        scalar2=cr_row, op0=OP.mult, op1=OP.add,
    )
    if stage <= 7:
        return dbg(init_acc)

    # ---- phase 3: per-group C_own, diff, dist2 ----
    dist2 = small.tile([P, CHUNKS], F32)
    for g in range(GROUPS):
        pg = p_2.tile([P, CPG * E], F32)
        nc.tensor.matmul(
            pg, lhsT=mskT[:, g, 0:P], rhs=c4bd, start=True, stop=True
        )
        diff_g = work.tile([P, CPG * E], F32)
        nc.vector.tensor_sub(diff_g, pg, emb_sb[:, g * CPG * E:(g + 1) * CPG * E])
        sq_g = work.tile([P, CPG * E], F32)
        nc.scalar.square(sq_g, diff_g)
        nc.vector.reduce_sum(
            out=dist2[:, g * CPG:(g + 1) * CPG],
            in_=sq_g.rearrange("p (a b) -> p a b", b=E),
            axis=AX.X,
        )
    if stage <= 8:
        return dbg(dist2)

    # ---- variance hinge + final reduction ----
    s_sb = small.tile([P, CHUNKS], F32)
    nc.scalar.sqrt(s_sb, dist2)
    hv_sb = small.tile([P, CHUNKS], F32)
    nc.scalar.activation(hv_sb, s_sb, AF.Relu, bias=bias_v, scale=1.0)
    scr_v = small.tile([P, CHUNKS], F32)
    tall_raw = small.tile([P, 1], F32)
    nc.vector.tensor_mul(scr_v, hv_sb, hv_sb)
    nc.vector.reduce_sum(out=tall_raw, in_=scr_v, axis=AX.X)
    tall = small.tile([P, 1], F32)
    nc.vector.tensor_scalar(
        out=tall, in0=tall_raw, scalar1=ALPHA / float(N),
        scalar2=init_acc, op0=OP.mult, op1=OP.add,
    )
    f_psum = p_sm.tile([1, 1], F32, tag="sm")
    nc.tensor.matmul(f_psum, lhsT=tall, rhs=ones1, start=True, stop=True)
    out_sb = small.tile([1, 1], F32)
    nc.scalar.copy(out_sb, f_psum)
    nc.sync.dma_start(out=t["out"][:, :], in_=out_sb)


def build_nc(stage=99):
    nc = bacc.Bacc("TRN2", target_bir_lowering=False, debug=False)
    t = {
        "emb": nc.dram_tensor("emb", [N, E], F32, kind="ExternalInput"),
        "msk": nc.dram_tensor("msk", [N, K], F32, kind="ExternalInput"),
        # [I_128 | ones]: transpose helper; last column produces row-sums
        "id129": nc.dram_tensor("id129", [P, P + 1], F32, kind="ExternalInput"),
        # stacked I_32 blocks: folds within-group msum partials
        "stki": nc.dram_tensor("stki", [P, K], F32, kind="ExternalInput"),
        # 1 - I_32: zeroes the pairwise-hinge diagonal
        "eyec": nc.dram_tensor("eyec", [K, K], F32, kind="ExternalInput"),
        "out": nc.dram_tensor("out", [1, 1], F32, kind="ExternalOutput"),
    }
    if stage < 99:
        t["dbg"] = nc.dram_tensor("dbg", [P, 2048], F32, kind="ExternalOutput")

    with tile.TileContext(nc) as tc, ExitStack() as ctx:
        _body(nc, tc, ctx, t, stage)

    nc.compile()
    return nc


def host_consts():
    id129 = np.concatenate(
        [np.eye(P, dtype=np.float32), np.ones((P, 1), np.float32)], axis=1
    )
    stki = np.tile(np.eye(K, dtype=np.float32), (CPG, 1))
    eyec = np.ones((K, K), np.float32) - np.eye(K, dtype=np.float32)
    return id129, stki, eyec


def make_in_maps(embedded, masks):
    emb = np.ascontiguousarray(np.asarray(embedded, dtype=np.float32))
    msk = np.ascontiguousarray(np.asarray(masks, dtype=np.float32))
    id129, stki, eyec = host_consts()
    return [
        {"emb": emb[i], "msk": msk[i], "id129": id129, "stki": stki, "eyec": eyec}
        for i in range(B)
    ]


_NC = None


def _get_nc():
    global _NC
    if _NC is None:
        _NC = build_nc()
    return _NC


def _install_ntff_shim():
    """Register the axon NTFF profile hook if the image's antenv lacks it."""
    import sys as _sys
    import types as _types

    try:
        from antenv.axon_hooks import get_axon_ntff_profile_hook  # noqa: F401
        return
    except ImportError:
        pass
    try:
        from trn_agent_boot.trn_boot import _ntff_profile_via_ctypes

        hook = _ntff_profile_via_ctypes("/opt/axon/libaxon_pjrt.so")
        mod = _types.ModuleType("antenv.axon_hooks")
        mod.get_axon_ntff_profile_hook = lambda: hook
        mod.set_axon_ntff_profile_hook = lambda h: None
        _sys.modules["antenv.axon_hooks"] = mod
    except Exception:
        pass


def run(embedded, masks, trace=False):
    nc = _get_nc()
    if trace:
        _install_ntff_shim()
    res = bass_utils.run_bass_kernel_spmd(
        nc, make_in_maps(embedded, masks), core_ids=list(range(B)), trace=trace
    )
    vals = np.array([r["out"][0, 0] for r in res.results], dtype=np.float64)
    return np.asarray(vals.mean(), dtype=np.float32), res


def kernel(embedded, masks, size):
    out, _ = run(embedded, masks)
    return out


# revision 17
# speedup vs baseline: 1.3370x; 1.3370x over previous
"""Trainium2 Bass kernel for a discriminative (instance-segmentation) loss.

Math (per batch b, with E=64-dim embeddings, K=32 clusters, N=4096 points):
  centroids C[k] = sum_n masks[n,k]*emb[n] / msum[k]
  L_v = mean_b sum_n relu(||emb_n - C_own(n)|| - 0.5)^2 / N
  L_d = mean_b sum_{k!=j} relu(3 - ||C_k - C_j||)^2 / (K*(K-1))
  L_r = mean_b mean_k ||C_k||
  loss = L_v + L_d + 0.001 * L_r

Sharding: data-parallel over the batch dim (B=8 -> 8 NeuronCores, one batch
each).  Each core computes its per-batch scalar; the host averages the 8
scalars.

Per-core layout: n = 32*p + c  (p = SBUF partition 0..127, c = chunk 0..31),
so each partition's slice of `emb`/`masks` is one contiguous DRAM block
(line-rate DMA descriptors).  Chunks are processed in 8 groups of 4.

Inputs are fed in bf16 (masks are exactly representable; emb rounding is
~1e-5 of the loss) which halves DMA bytes and runs the PE at 1 cycle/col
instead of fp32's 4.  All accumulation stays fp32 (PSUM + DVE/ACT).

Pipeline per core:
  1. PE: Cu^T[e,k]  = sum_c emb_c^T @ masks_c            (32 accum. matmuls)
  2. PE: masksT groups via matmul against [I_128 | 1]    (8 matmuls; the
     extra ones-column yields per-group cluster-count partials for msum)
  3. msum -> 1/msum; normalize Cu -> C; cn2[k] = ||C_k||^2
  4. PE: C_own per group via block-diag trick:
     lhsT = masksT_g [128,128], rhs = blockdiag(C,C,C,C) [128,256]
     -> psum[p, 64j+e] = C_own(n=32p+4g+j, e)
  5. DVE/ACT: diff = C_own - emb; dist2 = sum_e diff^2;
     hv = relu(sqrt(dist2)-0.5); ACT-square-accumulate hv^2/N
  6. tiny [32,32] centroid-pairwise hinge + mean-norm tail
  7. one final matmul against ones -> scalar -> DMA out

NOTE: InstTensorTensorReduce crashes the device on this path -- use
separate mul/square + reduce instead.
"""

from contextlib import ExitStack

import numpy as np
import ml_dtypes

import concourse.bass as bass
import concourse.bacc as bacc
import concourse.tile as tile
from concourse import mybir
from concourse import bass_utils

F32 = mybir.dt.float32
BF16 = mybir.dt.bfloat16
AX = mybir.AxisListType
OP = mybir.AluOpType
AF = mybir.ActivationFunctionType

B, N, E, K = 8, 4096, 64, 32
P = 128            # SBUF partitions; n = 32*p + c
CHUNKS = N // P    # 32
GROUPS = 8         # 4 chunks per group
CPG = CHUNKS // GROUPS  # 4
DELTA_V = 0.5
DELTA_D = 1.5
ALPHA, BETA, GAMMA = 1.0, 1.0, 0.001

# const pack columns (bf16): [I_128 | ones | stackedI_32 | (1 - I_32)]
CP_ID = 0          # id129: cols 0..128 inclusive of ones col
CP_STKI = P + 1    # 129..160
CP_EYEC = P + 1 + K  # 161..192
CP_W = P + 1 + 2 * K


def _body(nc, tc, ctx, t, stage):
    """Emit the kernel body. `stage` < 99 stops early and DMAs an
    intermediate to the debug output (bisection aid)."""
    consts = ctx.enter_context(tc.tile_pool(name="consts", bufs=1))
    big = ctx.enter_context(tc.tile_pool(name="big", bufs=1))
    work = ctx.enter_context(tc.tile_pool(name="work", bufs=3))
    small = ctx.enter_context(tc.tile_pool(name="small", bufs=1))
    p_cu = ctx.enter_context(tc.tile_pool(name="p_cu", bufs=1, space="PSUM"))
    p_mt = ctx.enter_context(tc.tile_pool(name="p_mt", bufs=2, space="PSUM"))
    p_2 = ctx.enter_context(tc.tile_pool(name="p_2", bufs=3, space="PSUM"))
    p_sm = ctx.enter_context(tc.tile_pool(name="p_sm", bufs=2, space="PSUM"))

    def dbg(ap):
        rows, cols = ap.shape[0], int(np.prod(ap.shape[1:]))
        flat = ap if len(ap.shape) == 2 else ap.rearrange("p ... -> p (...)")
        tmp = small.tile([rows, cols], F32, tag="dbgtmp")
        nc.scalar.copy(tmp, flat)
        nc.sync.dma_start(out=t["dbg"][0:rows, 0:cols], in_=tmp)

    # ---- constants / memsets ----
    cpack = consts.tile([P, CP_W], BF16)
    nc.scalar.dma_start(out=cpack, in_=t["cpack"][:, :])
    id129 = cpack[:, CP_ID:CP_ID + P + 1]
    stki = cpack[:, CP_STKI:CP_STKI + K]
    eyec = cpack[0:K, CP_EYEC:CP_EYEC + K]

    ones1 = consts.tile([P, 1], F32)
    nc.vector.memset(ones1, 1.0)
    bias_m = consts.tile([K, 1], F32)     # 2*DELTA_D margin bias
    nc.vector.memset(bias_m, 2.0 * DELTA_D)
    bias_v = consts.tile([P, 1], F32)     # -DELTA_V hinge bias
    nc.vector.memset(bias_v, -DELTA_V)
    c4bd = big.tile([P, CPG * E], BF16)   # blockdiag(C x4), filled later
    nc.vector.memset(c4bd, 0.0)
    init_acc = small.tile([P, 1], F32)    # hinge-sum init vector
    nc.vector.memset(init_acc, 0.0)

    # warm the ACT tables (Square/Sqrt/Relu) while DMAs stream
    warm = small.tile([1, 1], F32)
    nc.scalar.activation(warm, ones1[0:1, :], AF.Square)
    nc.scalar.activation(warm, ones1[0:1, :], AF.Sqrt)
    nc.scalar.activation(warm, ones1[0:1, :], AF.Relu)

    # ---- input loads: emb on the SP ring, masks on the ACT ring ----
    emb_sb = big.tile([P, CHUNKS * E], BF16)       # [p, 64*c + e]
    msk_sb = big.tile([P, CHUNKS, K], BF16)        # [p, c, k]
    nc.scalar.dma_start(
        out=msk_sb, in_=t["msk"][:, :].rearrange("(p c) k -> p c k", p=P)
    )
    nc.sync.dma_start(
        out=emb_sb, in_=t["emb"][:, :].rearrange("(p c) e -> p (c e)", p=P)
    )
    if stage <= 1:
        return dbg(msk_sb[:, 0:4, :])

    # ---- phase 1: masks transposes (+msum partials) ----
    mskT = big.tile([P, GROUPS, P + 1], BF16)  # [32j+k, g, p | msum partial]
    for g in range(GROUPS):
        mview = msk_sb[:, g * CPG:(g + 1) * CPG, :].rearrange("p a b -> p (a b)")
        pt = p_mt.tile([P, P + 1], F32)
        nc.tensor.matmul(pt, lhsT=mview, rhs=id129, start=True, stop=True)
        nc.vector.tensor_copy(out=mskT[:, g, :], in_=pt)
    if stage <= 2:
        return dbg(mskT[:, 0, :])

    # ---- Cu^T accumulation ----
    cu_psum = p_cu.tile([E, K], F32)
    for c in range(CHUNKS):
        nc.tensor.matmul(
            cu_psum,
            lhsT=emb_sb[:, c * E:(c + 1) * E],
            rhs=msk_sb[:, c, :],
            start=(c == 0),
            stop=(c == CHUNKS - 1),
        )
    cuT_sb = small.tile([E, K], BF16)
    nc.scalar.copy(out=cuT_sb, in_=cu_psum)
    if stage <= 3:
        return dbg(cuT_sb)

    # ---- msum -> recip ----
    msum_parts = mskT[:, :, P:P + 1].rearrange("p g o -> p (g o)")  # [P, 8]
    msum_big = small.tile([P, 1], BF16)   # counts <= ~200, exact in bf16
    with nc.allow_low_precision(reason="per-group counts are small ints"):
        nc.vector.reduce_sum(out=msum_big, in_=msum_parts, axis=AX.X)
    ms_psum = p_sm.tile([K, 1], F32, tag="sm")
    nc.tensor.matmul(ms_psum, lhsT=stki, rhs=msum_big, start=True, stop=True)
    recip = small.tile([K, 1], F32)
    nc.vector.reciprocal(recip, ms_psum)
    if stage <= 4:
        return dbg(recip)

    # ---- C = (Cu^T)^T * recip; cn2 ----
    c_psum = p_sm.tile([K, E], F32, tag="sm")
    nc.tensor.matmul(c_psum, lhsT=cuT_sb, rhs=id129[0:E, 0:E],
                     start=True, stop=True)
    c_sb = small.tile([K, E], F32)
    nc.vector.tensor_scalar_mul(c_sb, in0=c_psum, scalar1=recip)
    c_bf = small.tile([K, E], BF16)
    nc.scalar.copy(c_bf, c_sb)
    scr_ke = small.tile([K, E], F32)
    cn2 = small.tile([K, 1], F32)
    nc.vector.tensor_mul(scr_ke, c_sb, c_sb)
    nc.vector.reduce_sum(out=cn2, in_=scr_ke, axis=AX.X)
    if stage <= 5:
        return dbg(c_sb)

    # ---- block-diag C (4 small SBUF->SBUF DMAs on the ACT ring) ----
    for j in range(CPG):
        nc.scalar.dma_start(
            out=c4bd[j * K:(j + 1) * K, j * E:(j + 1) * E], in_=c_bf
        )
    if stage <= 6:
        return dbg(c4bd)

    # ---- tiny pairwise-centroid tail (L_d, L_r) ----
    gu_psum = p_sm.tile([K, K], F32, tag="sm")
    nc.tensor.matmul(gu_psum, lhsT=cuT_sb, rhs=cuT_sb, start=True, stop=True)
    recipm2 = small.tile([K, 1], F32)
    nc.scalar.mul(recipm2, recip, -2.0)
    x_sb = small.tile([K, K], BF16)
    nc.vector.tensor_scalar_mul(x_sb, in0=gu_psum, scalar1=recipm2)
    xt_psum = p_sm.tile([K, K], F32, tag="sm")
    nc.tensor.matmul(xt_psum, lhsT=x_sb, rhs=id129[0:K, 0:K],
                     start=True, stop=True)
    z_sb = small.tile([K, K], BF16)
    nc.vector.tensor_scalar(
        out=z_sb, in0=xt_psum, scalar1=recip, scalar2=cn2,
        op0=OP.mult, op1=OP.add,
    )
    zt_psum = p_sm.tile([K, K], F32, tag="sm")
    nc.tensor.matmul(zt_psum, lhsT=z_sb, rhs=id129[0:K, 0:K],
                     start=True, stop=True)
    d2_sb = small.tile([K, K], F32)
    nc.vector.tensor_scalar(
        out=d2_sb, in0=zt_psum, scalar1=cn2, scalar2=0.0,
        op0=OP.add, op1=OP.max,
    )
    d_sb = small.tile([K, K], F32)
    nc.scalar.sqrt(d_sb, d2_sb)
    h0_sb = small.tile([K, K], F32)
    nc.scalar.activation(h0_sb, d_sb, AF.Relu, bias=bias_m, scale=-1.0)
    h_sb = small.tile([K, K], F32)
    nc.vector.tensor_mul(h_sb, h0_sb, eyec)
    scr_kk = small.tile([K, K], F32)
    ld_raw = small.tile([K, 1], F32)
    nc.scalar.activation(scr_kk, h_sb, AF.Square, accum_out=ld_raw)
    cr_row = small.tile([K, 1], F32)
    nc.scalar.activation(cr_row, cn2, AF.Sqrt, scale=(GAMMA / K) ** 2)
    nc.vector.tensor_scalar(
        out=init_acc[0:K, :], in0=ld_raw, scalar1=BETA / float(K * (K - 1)),
        scalar2=cr_row, op0=OP.mult, op1=OP.add,
    )
    if stage <= 7:
        return dbg(init_acc)

    # ---- phase 3: per-group C_own, diff, dist2 ----
    dist2 = small.tile([P, CHUNKS], F32)
    for g in range(GROUPS):
        pg = p_2.tile([P, CPG * E], F32)
        nc.tensor.matmul(
            pg, lhsT=mskT[:, g, 0:P], rhs=c4bd, start=True, stop=True
        )
        diff_g = work.tile([P, CPG * E], F32)
        nc.vector.tensor_sub(diff_g, pg, emb_sb[:, g * CPG * E:(g + 1) * CPG * E])
        sq_g = work.tile([P, CPG * E], F32)
        nc.scalar.square(sq_g, diff_g)
        nc.vector.reduce_sum(
            out=dist2[:, g * CPG:(g + 1) * CPG],
            in_=sq_g.rearrange("p (a b) -> p a b", b=E),
            axis=AX.X,
        )
    if stage <= 8:
        return dbg(dist2)

    # ---- variance hinge + final reduction ----
    s_sb = small.tile([P, CHUNKS], F32)
    nc.scalar.sqrt(s_sb, dist2)
    hv_sb = small.tile([P, CHUNKS], F32)
    nc.scalar.activation(hv_sb, s_sb, AF.Relu, bias=bias_v, scale=1.0)
    scr_v = small.tile([P, CHUNKS], F32)
    tall_raw = small.tile([P, 1], F32)
    nc.scalar.activation(scr_v, hv_sb, AF.Square, accum_out=tall_raw)
    tall = small.tile([P, 1], F32)
    nc.vector.tensor_scalar(
        out=tall, in0=tall_raw, scalar1=ALPHA / float(N),
        scalar2=init_acc, op0=OP.mult, op1=OP.add,
    )
    f_psum = p_sm.tile([1, 1], F32, tag="sm")
    nc.tensor.matmul(f_psum, lhsT=tall, rhs=ones1, start=True, stop=True)
    out_sb = small.tile([1, 1], F32)
    nc.scalar.copy(out_sb, f_psum)
    nc.sync.dma_start(out=t["out"][:, :], in_=out_sb)


def build_nc(stage=99):
    nc = bacc.Bacc("TRN2", target_bir_lowering=False, debug=False)
    t = {
        "emb": nc.dram_tensor("emb", [N, E], BF16, kind="ExternalInput"),
        "msk": nc.dram_tensor("msk", [N, K], BF16, kind="ExternalInput"),
        "cpack": nc.dram_tensor("cpack", [P, CP_W], BF16, kind="ExternalInput"),
        "out": nc.dram_tensor("out", [1, 1], F32, kind="ExternalOutput"),
    }
    if stage < 99:
        t["dbg"] = nc.dram_tensor("dbg", [P, 2048], F32, kind="ExternalOutput")

    with tile.TileContext(nc) as tc, ExitStack() as ctx:
        _body(nc, tc, ctx, t, stage)

    nc.compile()
    return nc


def host_consts():
    cpack = np.zeros((P, CP_W), dtype=ml_dtypes.bfloat16)
    cpack[:, 0:P] = np.eye(P)
    cpack[:, P] = 1.0
    cpack[:, CP_STKI:CP_STKI + K] = np.tile(np.eye(K), (CPG, 1))
    cpack[0:K, CP_EYEC:CP_EYEC + K] = 1.0 - np.eye(K)
    return cpack


def make_in_maps(embedded, masks):
    emb = np.asarray(embedded).astype(ml_dtypes.bfloat16)
    msk = np.asarray(masks).astype(ml_dtypes.bfloat16)
    cpack = host_consts()
    return [
        {"emb": np.ascontiguousarray(emb[i]),
         "msk": np.ascontiguousarray(msk[i]),
         "cpack": cpack}
        for i in range(B)
    ]


_NC = None


def _get_nc():
    global _NC
    if _NC is None:
        _NC = build_nc()
    return _NC


def _install_ntff_shim():
    """Register the axon NTFF profile hook if the image's antenv lacks it."""
    import sys as _sys
    import types as _types

    try:
        from antenv.axon_hooks import get_axon_ntff_profile_hook  # noqa: F401
        return
    except ImportError:
        pass
    try:
        from trn_agent_boot.trn_boot import _ntff_profile_via_ctypes

        hook = _ntff_profile_via_ctypes("/opt/axon/libaxon_pjrt.so")
        mod = _types.ModuleType("antenv.axon_hooks")
        mod.get_axon_ntff_profile_hook = lambda: hook
        mod.set_axon_ntff_profile_hook = lambda h: None
        _sys.modules["antenv.axon_hooks"] = mod
    except Exception:
        pass


def run(embedded, masks, trace=False):
    nc = _get_nc()
    if trace:
        _install_ntff_shim()
    res = bass_utils.run_bass_kernel_spmd(
        nc, make_in_maps(embedded, masks), core_ids=list(range(B)), trace=trace
    )
    vals = np.array([r["out"][0, 0] for r in res.results], dtype=np.float64)
    return np.asarray(vals.mean(), dtype=np.float32), res


def kernel(embedded, masks, size):
    out, _ = run(embedded, masks)
    return out


# revision 22
# speedup vs baseline: 1.5297x; 1.1442x over previous
"""Trainium2 Bass kernel for a discriminative (instance-segmentation) loss.

Math (per batch b, with E=64-dim embeddings, K=32 clusters, N=4096 points):
  centroids C[k] = sum_n masks[n,k]*emb[n] / msum[k]
  L_v = mean_b sum_n relu(||emb_n - C_own(n)|| - 0.5)^2 / N
  L_d = mean_b sum_{k!=j} relu(3 - ||C_k - C_j||)^2 / (K*(K-1))
  L_r = mean_b mean_k ||C_k||
  loss = L_v + L_d + 0.001 * L_r

Sharding: data-parallel over the batch dim (B=8 -> 8 NeuronCores, one batch
each).  Each core computes its per-batch scalar; the host averages the 8
scalars.

Per-core layout: n = 32*p + c  (p = SBUF partition 0..127, c = chunk 0..31),
so each partition's slice of `emb`/`masks` is one contiguous DRAM block
(line-rate DMA descriptors).  Chunks are processed in 8 groups of 4.

Inputs are fed in bf16 (masks are exactly representable; emb rounding is
~1e-5 of the loss) which halves DMA bytes and runs the PE at 1 cycle/col
instead of fp32's 4.  All accumulation stays fp32 (PSUM + DVE/ACT).

Pipeline per core:
  1. PE: Cu^T[e,k]  = sum_c emb_c^T @ masks_c            (32 accum. matmuls)
  2. PE: masksT groups via matmul against [I_128 | 1]    (8 matmuls; the
     extra ones-column yields per-group cluster-count partials for msum)
  3. msum -> 1/msum; normalize Cu -> C; cn2[k] = ||C_k||^2
  4. PE: C_own per group via block-diag trick:
     lhsT = masksT_g [128,128], rhs = blockdiag(C,C,C,C) [128,256]
     -> psum[p, 64j+e] = C_own(n=32p+4g+j, e)
  5. DVE/ACT: diff = C_own - emb; dist2 = sum_e diff^2;
     hv = relu(sqrt(dist2)-0.5); ACT-square-accumulate hv^2/N
  6. tiny [32,32] centroid-pairwise hinge + mean-norm tail
  7. one final matmul against ones -> scalar -> DMA out

NOTE: InstTensorTensorReduce crashes the device on this path -- use
separate mul/square + reduce instead.
"""

from contextlib import ExitStack

import numpy as np
import ml_dtypes

import concourse.bass as bass
import concourse.bacc as bacc
import concourse.tile as tile
from concourse import mybir
from concourse import bass_utils

F32 = mybir.dt.float32
BF16 = mybir.dt.bfloat16
AX = mybir.AxisListType
OP = mybir.AluOpType
AF = mybir.ActivationFunctionType

B, N, E, K = 8, 4096, 64, 32
P = 128            # SBUF partitions; n = 32*p + c
CHUNKS = N // P    # 32
GROUPS = 8         # 4 chunks per group
CPG = CHUNKS // GROUPS  # 4
DELTA_V = 0.5
DELTA_D = 1.5
ALPHA, BETA, GAMMA = 1.0, 1.0, 0.001

# const pack columns (bf16): [I_128 | ones | stackedI_32 | (1 - I_32) | stackedI^T]
CP_ID = 0          # id129: cols 0..128 inclusive of ones col
CP_STKI = P + 1    # 129..160
CP_EYEC = P + 1 + K  # 161..192
CP_STKIT = P + 1 + 2 * K  # 193..320 (rows 0:32 valid)
CP_W = P + 1 + 2 * K + P


def _body(nc, tc, ctx, t, stage):
    """Emit the kernel body. `stage` < 99 stops early and DMAs an
    intermediate to the debug output (bisection aid)."""
    consts = ctx.enter_context(tc.tile_pool(name="consts", bufs=1))
    big = ctx.enter_context(tc.tile_pool(name="big", bufs=1))
    work = ctx.enter_context(tc.tile_pool(name="work", bufs=3))
    small = ctx.enter_context(tc.tile_pool(name="small", bufs=1))
    p_cu = ctx.enter_context(tc.tile_pool(name="p_cu", bufs=1, space="PSUM"))
    p_mt = ctx.enter_context(tc.tile_pool(name="p_mt", bufs=2, space="PSUM"))
    p_2 = ctx.enter_context(tc.tile_pool(name="p_2", bufs=3, space="PSUM"))
    p_sm = ctx.enter_context(tc.tile_pool(name="p_sm", bufs=2, space="PSUM"))

    def dbg(ap):
        rows, cols = ap.shape[0], int(np.prod(ap.shape[1:]))
        flat = ap if len(ap.shape) == 2 else ap.rearrange("p ... -> p (...)")
        tmp = small.tile([rows, cols], F32, tag="dbgtmp")
        nc.scalar.copy(tmp, flat)
        nc.sync.dma_start(out=t["dbg"][0:rows, 0:cols], in_=tmp)

    # ---- constants / memsets ----
    cpack = consts.tile([P, CP_W], BF16)
    nc.scalar.dma_start(out=cpack, in_=t["cpack"][:, :])
    id129 = cpack[:, CP_ID:CP_ID + P + 1]
    stki = cpack[:, CP_STKI:CP_STKI + K]
    eyec = cpack[0:K, CP_EYEC:CP_EYEC + K]
    stkit = cpack[0:K, CP_STKIT:CP_STKIT + P]

    ones1 = consts.tile([P, 1], F32)
    nc.vector.memset(ones1, 1.0)
    bias_m = consts.tile([K, 1], F32)     # 2*DELTA_D margin bias
    nc.vector.memset(bias_m, 2.0 * DELTA_D)
    bias_v = consts.tile([P, 1], F32)     # -DELTA_V hinge bias
    nc.vector.memset(bias_v, -DELTA_V)
    c4bd = big.tile([P, CPG * E], BF16)   # blockdiag(C x4), filled later
    nc.vector.memset(c4bd, 0.0)
    init_acc = small.tile([P, 1], F32)    # hinge-sum init vector
    nc.vector.memset(init_acc, 0.0)

    # warm the ACT tables (Square/Sqrt/Relu) while DMAs stream
    warm = small.tile([1, 1], F32)
    nc.scalar.activation(warm, ones1[0:1, :], AF.Square)
    nc.scalar.activation(warm, ones1[0:1, :], AF.Sqrt)
    nc.scalar.activation(warm, ones1[0:1, :], AF.Relu)

    # ---- input loads: emb on the SP ring, masks on the ACT ring ----
    emb_sb = big.tile([P, CHUNKS * E], BF16)       # [p, 64*c + e]
    msk_sb = big.tile([P, CHUNKS, K], BF16)        # [p, c, k]
    nc.scalar.dma_start(
        out=msk_sb, in_=t["msk"][:, :].rearrange("(p c) k -> p c k", p=P)
    )
    nc.sync.dma_start(
        out=emb_sb, in_=t["emb"][:, :].rearrange("(p c) e -> p (c e)", p=P)
    )
    if stage <= 1:
        return dbg(msk_sb[:, 0:4, :])

    # ---- phase 1: masks transposes (+msum partials) ----
    mskT = big.tile([P, GROUPS, P + 1], BF16)  # [32j+k, g, p | msum partial]
    for g in range(GROUPS):
        mview = msk_sb[:, g * CPG:(g + 1) * CPG, :].rearrange("p a b -> p (a b)")
        pt = p_mt.tile([P, P + 1], F32)
        nc.tensor.matmul(pt, lhsT=mview, rhs=id129, start=True, stop=True)
        nc.vector.tensor_copy(out=mskT[:, g, :], in_=pt)
    if stage <= 2:
        return dbg(mskT[:, 0, :])

    # ---- Cu^T accumulation: 16 chunk-pair matmuls ----
    # lhsT = [emb_2i | emb_2i+1] [128,128], rhs = [msk_2i | msk_2i+1] [128,64]
    # -> psum [128,64]; wanted sums live in the TL [0:64,0:32] and
    # BR [64:128,32:64] blocks (cross blocks are ignored).
    cu_psum = p_cu.tile([P, 2 * K], F32)
    NP = CHUNKS // 2
    for i in range(NP):
        nc.tensor.matmul(
            cu_psum,
            lhsT=emb_sb[:, i * 2 * E:(i + 1) * 2 * E],
            rhs=msk_sb[:, 2 * i:2 * i + 2, :].rearrange("p a b -> p (a b)"),
            start=(i == 0),
            stop=(i == NP - 1),
        )
    cuT_sb = small.tile([P, 2 * K], BF16)
    nc.scalar.copy(out=cuT_sb, in_=cu_psum)
    if stage <= 3:
        return dbg(cuT_sb)

    # ---- msum -> recip ----
    msum_parts = mskT[:, :, P:P + 1].rearrange("p g o -> p (g o)")  # [P, 8]
    msum_big = small.tile([P, 1], BF16)   # counts <= ~200, exact in bf16
    with nc.allow_low_precision(reason="per-group counts are small ints"):
        nc.vector.reduce_sum(out=msum_big, in_=msum_parts, axis=AX.X)
    ms_psum = p_sm.tile([K, 1], F32, tag="sm")
    nc.tensor.matmul(ms_psum, lhsT=stki, rhs=msum_big, start=True, stop=True)
    recip = small.tile([K, 1], F32)
    nc.vector.reciprocal(recip, ms_psum)
    if stage <= 4:
        return dbg(recip)

    # ---- C = (Cu^T)^T * recip; cn2 ----
    # Both transposes keep base partition 0 (quadrant-offset PE operands
    # crash the device): full-height lhsT, row-selection via shifted
    # identity columns ([I64;0] then [0;I64]).
    c_psum = p_sm.tile([K, E], F32, tag="sm")
    nc.tensor.matmul(c_psum, lhsT=cuT_sb[:, 0:K], rhs=id129[:, 0:E],
                     start=True, stop=False)
    nc.tensor.matmul(c_psum, lhsT=cuT_sb[:, K:2 * K], rhs=id129[:, E:P],
                     start=False, stop=True)
    if stage == 44:
        return dbg(c_psum)
    c_bf = small.tile([K, E], BF16)
    nc.vector.tensor_scalar_mul(c_bf, in0=c_psum, scalar1=recip)
    if stage == 45:
        return dbg(c_bf)
    scr_ke = small.tile([K, E], F32)
    cn2 = small.tile([K, 1], F32)
    nc.vector.tensor_mul(scr_ke, c_bf, c_bf)
    nc.vector.reduce_sum(out=cn2, in_=scr_ke, axis=AX.X)
    if stage <= 5:
        return dbg(c_bf)

    # ---- block-diag C: PE-replicate C 4x vertically, then 4 lane-aligned
    # copies into the diagonal blocks (partition ranges match, no DMA).
    rep_psum = p_sm.tile([P, E], F32, tag="sm")
    nc.tensor.matmul(rep_psum, lhsT=stkit, rhs=c_bf, start=True, stop=True)
    for j in range(CPG):
        dst = c4bd[j * K:(j + 1) * K, j * E:(j + 1) * E]
        src = rep_psum[j * K:(j + 1) * K, :]
        if j % 2 == 0:
            nc.vector.tensor_copy(out=dst, in_=src)
        else:
            nc.scalar.copy(out=dst, in_=src)
    if stage <= 6:
        return dbg(c4bd)

    # ---- tiny pairwise-centroid tail (L_d, L_r) ----
    ct_psum = p_sm.tile([E, K], F32, tag="sm")
    nc.tensor.matmul(ct_psum, lhsT=c_bf, rhs=id129[0:K, 0:K],
                     start=True, stop=True)
    ct_sb = small.tile([E, K], BF16)
    nc.scalar.copy(ct_sb, ct_psum)
    g_psum = p_sm.tile([K, K], F32, tag="sm")
    nc.tensor.matmul(g_psum, lhsT=ct_sb, rhs=ct_sb, start=True, stop=True)
    w_sb = small.tile([K, K], BF16)
    nc.vector.tensor_scalar(
        out=w_sb, in0=g_psum, scalar1=-2.0, scalar2=cn2,
        op0=OP.mult, op1=OP.add,
    )
    wt_psum = p_sm.tile([K, K], F32, tag="sm")
    nc.tensor.matmul(wt_psum, lhsT=w_sb, rhs=id129[0:K, 0:K],
                     start=True, stop=True)
    d2_sb = small.tile([K, K], F32)
    nc.vector.tensor_scalar(
        out=d2_sb, in0=wt_psum, scalar1=cn2, scalar2=0.0,
        op0=OP.add, op1=OP.max,
    )
    d_sb = small.tile([K, K], F32)
    nc.scalar.sqrt(d_sb, d2_sb)
    h0_sb = small.tile([K, K], F32)
    nc.scalar.activation(h0_sb, d_sb, AF.Relu, bias=bias_m, scale=-1.0)
    h_sb = small.tile([K, K], F32)
    nc.vector.tensor_mul(h_sb, h0_sb, eyec)
    scr_kk = small.tile([K, K], F32)
    ld_raw = small.tile([K, 1], F32)
    nc.scalar.activation(scr_kk, h_sb, AF.Square, accum_out=ld_raw)
    cr_row = small.tile([K, 1], F32)
    nc.scalar.activation(cr_row, cn2, AF.Sqrt, scale=(GAMMA / K) ** 2)
    nc.vector.tensor_scalar(
        out=init_acc[0:K, :], in0=ld_raw, scalar1=BETA / float(K * (K - 1)),
        scalar2=cr_row, op0=OP.mult, op1=OP.add,
    )
    if stage <= 7:
        return dbg(init_acc)

    # ---- phase 3: per-group C_own, diff, dist2 ----
    dist2 = small.tile([P, CHUNKS], F32)
    for g in range(GROUPS):
        pg = p_2.tile([P, CPG * E], F32)
        nc.tensor.matmul(
            pg, lhsT=mskT[:, g, 0:P], rhs=c4bd, start=True, stop=True
        )
        diff_g = work.tile([P, CPG * E], F32)
        nc.vector.tensor_sub(diff_g, pg, emb_sb[:, g * CPG * E:(g + 1) * CPG * E])
        sq_g = work.tile([P, CPG * E], BF16)
        nc.scalar.square(sq_g, diff_g)
        nc.vector.reduce_sum(
            out=dist2[:, g * CPG:(g + 1) * CPG],
            in_=sq_g.rearrange("p (a b) -> p a b", b=E),
            axis=AX.X,
        )
    if stage <= 8:
        return dbg(dist2)

    # ---- variance hinge + final reduction ----
    s_sb = small.tile([P, CHUNKS], F32)
    nc.scalar.sqrt(s_sb, dist2)
    hv_sb = small.tile([P, CHUNKS], F32)
    nc.scalar.activation(hv_sb, s_sb, AF.Relu, bias=bias_v, scale=1.0)
    scr_v = small.tile([P, CHUNKS], F32)
    tall_raw = small.tile([P, 1], F32)
    nc.scalar.activation(scr_v, hv_sb, AF.Square, accum_out=tall_raw)
    tall = small.tile([P, 1], F32)
    nc.vector.tensor_scalar(
        out=tall, in0=tall_raw, scalar1=ALPHA / float(N),
        scalar2=init_acc, op0=OP.mult, op1=OP.add,
    )
    f_psum = p_sm.tile([1, 1], F32, tag="sm")
    nc.tensor.matmul(f_psum, lhsT=tall, rhs=ones1, start=True, stop=True)
    out_sb = small.tile([1, 1], F32)
    nc.scalar.copy(out_sb, f_psum)
    nc.sync.dma_start(out=t["out"][:, :], in_=out_sb)


def build_nc(stage=99):
    nc = bacc.Bacc("TRN2", target_bir_lowering=False, debug=False)
    t = {
        "emb": nc.dram_tensor("emb", [N, E], BF16, kind="ExternalInput"),
        "msk": nc.dram_tensor("msk", [N, K], BF16, kind="ExternalInput"),
        "cpack": nc.dram_tensor("cpack", [P, CP_W], BF16, kind="ExternalInput"),
        "out": nc.dram_tensor("out", [1, 1], F32, kind="ExternalOutput"),
    }
    if stage < 99:
        t["dbg"] = nc.dram_tensor("dbg", [P, 2048], F32, kind="ExternalOutput")

    with tile.TileContext(nc) as tc, ExitStack() as ctx:
        _body(nc, tc, ctx, t, stage)

    nc.compile()
    return nc


def host_consts():
    cpack = np.zeros((P, CP_W), dtype=ml_dtypes.bfloat16)
    cpack[:, 0:P] = np.eye(P)
    cpack[:, P] = 1.0
    cpack[:, CP_STKI:CP_STKI + K] = np.tile(np.eye(K), (CPG, 1))
    cpack[0:K, CP_EYEC:CP_EYEC + K] = 1.0 - np.eye(K)
    cpack[0:K, CP_STKIT:CP_STKIT + P] = np.tile(np.eye(K), (1, CPG))
    return cpack


def make_in_maps(embedded, masks):
    emb = np.asarray(embedded).astype(ml_dtypes.bfloat16)
    msk = np.asarray(masks).astype(ml_dtypes.bfloat16)
    cpack = host_consts()
    return [
        {"emb": np.ascontiguousarray(emb[i]),
         "msk": np.ascontiguousarray(msk[i]),
         "cpack": cpack}
        for i in range(B)
    ]


_NC = None


def _get_nc():
    global _NC
    if _NC is None:
        _NC = build_nc()
    return _NC


def _install_ntff_shim():
    """Register the axon NTFF profile hook if the image's antenv lacks it."""
    import sys as _sys
    import types as _types

    try:
        from antenv.axon_hooks import get_axon_ntff_profile_hook  # noqa: F401
        return
    except ImportError:
        pass
    try:
        from trn_agent_boot.trn_boot import _ntff_profile_via_ctypes

        hook = _ntff_profile_via_ctypes("/opt/axon/libaxon_pjrt.so")
        mod = _types.ModuleType("antenv.axon_hooks")
        mod.get_axon_ntff_profile_hook = lambda: hook
        mod.set_axon_ntff_profile_hook = lambda h: None
        _sys.modules["antenv.axon_hooks"] = mod
    except Exception:
        pass


def run(embedded, masks, trace=False):
    nc = _get_nc()
    if trace:
        _install_ntff_shim()
    res = bass_utils.run_bass_kernel_spmd(
        nc, make_in_maps(embedded, masks), core_ids=list(range(B)), trace=trace
    )
    vals = np.array([r["out"][0, 0] for r in res.results], dtype=np.float64)
    return np.asarray(vals.mean(), dtype=np.float32), res


def kernel(embedded, masks, size):
    out, _ = run(embedded, masks)
    return out
